# revision 15
# baseline (speedup 1.0000x reference)
"""Group-wise correlation cost volume (build_gwc_volume) on 8 trn2 cores.

volume[b,g,d,h,w] = sum_c ref[b,g,c,h,w] * tgt[b,g,c,h,w-d]  (0 where w<d)

Sharding: 16 (b,g) pairs across 8 cores, 2 pairs per core. Each pair is a
contiguous 64-channel slice of the inputs and a contiguous [D,H,W] slab of
the output.

Per (b,g,h) the volume rows are diagonals of the Gram matrix
G[w',w] = sum_c tgt[c,w'] * ref[c,w].  Only the band d = w - w' in [0,48)
is needed, so the Gram is computed as 8 column-piece matmuls (w' pieces of
32), each with an 80-wide moving window R[:, BASE_k : BASE_k+80) written
at a fixed offset of a PSUM bank.

Both (b,g) pairs are fused into ONE K=128 matmul per piece with
block-diagonal zero-padded weights: lhsT[128, 64] holds pair0's 32
tgt-columns in rows 0:64 / cols 0:32 and pair1's in rows 64:128 /
cols 32:64, zeros elsewhere, so the cross-pair products vanish.  The
moving operand is the natural 128-partition ref tile (both pairs
stacked), shared by the two pairs since the window depends only on the
piece.  This halves MATMUL+LDWEIGHTS instruction counts vs per-pair
K=64 matmuls — the weight-load path (one 128-row column per cycle) is
the tensor engine's serial resource for this shape.  The interleaved
weights are built on DVE as uint32 moves; the zero halves live in
persistent SBUF buffers memset once at kernel start.

Two h rows share one 2-bank PSUM tile so each PSUM->SBUF evacuation
instruction covers 2 h of band (amortizes the ~150ns fixed cost per
copy); the evacuation is split 150/170 columns across DVE/ACT to match
their measured per-column rates.  Output DMA rides the otherwise-idle
GPSIMD (SWDGE) queue, input DMA on sync (HWDGE).

Everything runs in bf16 (inputs rounded on the host, PSUM fp32
accumulation, band tiles stored back as bf16): rel err vs the fp32
reference is ~4e-3, well under the 2e-2 gate.

Diagonal (shear) extraction at 1-partition granularity is not expressible
in any engine's access patterns, so the 80-wide band tiles are DMAed out
and the diagonals are gathered on the host during unsharding.
"""

import sys

if "/opt/trn_rl_repo" not in sys.path:
    sys.path.insert(0, "/opt/trn_rl_repo")

import numpy as np
import ml_dtypes

import concourse.bacc as bacc
import concourse.tile as tile
from concourse import mybir
from concourse.bass_utils import run_bass_kernel_spmd

F32 = mybir.dt.float32
BF16 = mybir.dt.bfloat16
U32 = mybir.dt.uint32

B, C, H, W = 2, 512, 128, 256
G, CG, D = 8, 64, 48
N_CORES = 8
PAIRS = 2  # (b,g) pairs per core
HC = 16  # h rows per chunk
PW = 80  # piece window width (32 + 47 + 1)
NW = 4  # persistent interleaved-weight buffers (chunk ring)
XSPLIT = 96  # evacuation column split between DVE [0:XSPLIT] and ACT

# piece k covers w' in [32k, 32k+32); its moving window starts at
# BASE[k] = min(32k, W - PW) so every piece is a full 80 columns.
BASE = [min(32 * k, W - PW) for k in range(8)]

_cached = {}


def _build_module():
    nc = bacc.Bacc("TRN2", target_bir_lowering=False, debug=False, num_devices=N_CORES)
    # io[p, h, 0] = ref channel p, io[p, h, 1] = tgt channel p  (p = pair*64+c)
    io = nc.dram_tensor("io", [128, H, 2, W], BF16, kind="ExternalInput")
    # band tiles, layout [row, h, x]: row p = 64*(k%2) + 32*pair + (w'%32),
    # col x = 80*(k//2) + (w - BASE[k]) for piece k = w'//32.
    out_bt = nc.dram_tensor("out_bt", [128, H, 4 * PW], BF16, kind="ExternalOutput")

    with tile.TileContext(nc) as tc:
        with (
            tc.tile_pool(name="ins", bufs=5) as ins,
            tc.tile_pool(name="wts", bufs=1) as wts,
            tc.tile_pool(name="stage", bufs=3) as stage_pool,
            tc.tile_pool(name="psum", bufs=2, space="PSUM") as psum,
        ):
            # persistent zero-padded weight buffers: the zero halves are
            # written once here and never touched again (the per-chunk
            # builds only overwrite the data sub-blocks).
            warm = wts.tile([1, 8], BF16, tag="warm", name="warm", bufs=1)
            nc.scalar.copy(warm[0:1, 0:4], warm[0:1, 4:8])

            wt_bufs = []
            for i in range(NW):
                wtb = wts.tile(
                    [128, HC, 8, 64], BF16, tag=f"wt{i}", name=f"wt{i}", bufs=1
                )
                wt_bufs.append(wtb)

            for ch in range(H // HC):
                h0 = ch * HC
                it = ins.tile([128, HC, 2, W], BF16, tag="io", name=f"it{ch}")
                wt = wt_bufs[ch % NW]
                # chunk 0 is loaded/interleaved in 4h slivers so the first
                # matmuls start ~7us earlier (input buffers recycle sooner,
                # keeping the input DMA stream gap-free mid-kernel).
                nsub = 4 if ch == 0 else 1
                hs = HC // nsub
                for q in range(nsub):
                    hq = q * hs
                    nc.sync.dma_start(
                        it[:, hq : hq + hs], io[:, h0 + hq : h0 + hq + hs, :, :]
                    )
                    # zero-prime each persistent weight buffer sliver just
                    # before its first use (a single up-front prime pass
                    # would queue ~9us of DVE memsets ahead of the first
                    # interleave builds and delay the first matmul)
                    if ch < NW:
                        nc.vector.memset(wt[:, hq : hq + hs].bitcast(U32), 0)
                    # interleave tgt into the block-diagonal weight layout
                    # (bf16 pairs moved as uint32 for 2x DVE throughput)
                    tgt_lo = (
                        it[0:64, hq : hq + hs, 1]
                        .rearrange("p hl (k m) -> p hl k m", k=8)
                        .bitcast(U32)
                    )
                    tgt_hi = (
                        it[64:128, hq : hq + hs, 1]
                        .rearrange("p hl (k m) -> p hl k m", k=8)
                        .bitcast(U32)
                    )
                    nc.vector.tensor_copy(
                        wt[0:64, hq : hq + hs, :, 0:32].bitcast(U32), tgt_lo
                    )
                    nc.vector.tensor_copy(
                        wt[64:128, hq : hq + hs, :, 32:64].bitcast(U32), tgt_hi
                    )

                st = stage_pool.tile(
                    [128, HC, 4 * PW], BF16, tag="st", name=f"st{ch}"
                )
                for hh in range(HC // 4):
                    # one 4-bank PSUM tile holds four h rows of band
                    bank = psum.tile(
                        [128, 4, 512], F32, tag="bk", name=f"bk{ch}_{hh}"
                    )
                    for hl2 in range(4):
                        hl = 4 * hh + hl2
                        for k in range(8):
                            m0 = 64 * (k % 2)
                            c0 = PW * (k // 2)
                            nc.tensor.matmul(
                                bank[m0 : m0 + 64, hl2, c0 : c0 + PW],
                                wt[:, hl, k, :],
                                it[:, hl, 0, BASE[k] : BASE[k] + PW],
                                tile_position=(0, m0),
                            )
                    h2 = 4 * hh
                    nc.vector.tensor_copy(
                        st[:, h2 : h2 + 4, 0:XSPLIT], bank[:, :, 0:XSPLIT]
                    )
                    nc.scalar.copy(
                        st[:, h2 : h2 + 4, XSPLIT : 4 * PW],
                        bank[:, :, XSPLIT : 4 * PW],
                    )
                    if hh % 2 == 1:
                        # flush each 8h half as soon as its copies land —
                        # smooths the output stream and shortens the drain
                        hf = 4 * (hh - 1)
                        nc.scalar.dma_start(
                            out_bt[:, h0 + hf : h0 + hf + 8, :],
                            st[:, hf : hf + 8],
                        )

    nc.compile()
    return nc


def _get_module():
    if "nc" not in _cached:
        _cached["nc"] = _build_module()
    return _cached["nc"]


def _to_bf16(a):
    """Round-to-nearest-even fp32 -> bf16 without ml_dtypes' slow cast."""
    u = np.ascontiguousarray(a, np.float32).view(np.uint32)
    r = (u + np.uint32(0x7FFF) + ((u >> np.uint32(16)) & np.uint32(1))) >> np.uint32(16)
    return r.astype(np.uint16).view(ml_dtypes.bfloat16)


def _make_in_maps(refimg_fea, targetimg_fea):
    rp = _to_bf16(refimg_fea).reshape(N_CORES, 128, H, W)
    tp = _to_bf16(targetimg_fea).reshape(N_CORES, 128, H, W)
    io = np.ascontiguousarray(np.stack([rp, tp], axis=3))  # [8, 128, H, 2, W]
    return [{"io": io[k]} for k in range(N_CORES)]


def _host_extract(bt):
    """Gather band diagonals into the full volume.

    bt: [8, 128, H, 320] bf16 per core.  For disparity d at width w
    (valid when w >= d): w' = w - d, piece k = w'//32, row
    p = 64*(k%2) + 32*pair_local + (w'%32), col x = 80*(k//2) +
    (w - BASE[k]).
    """
    d = np.arange(D)[:, None]
    w = np.arange(W)[None, :]
    wp = w - d  # [D, W] source w' (negative -> zero region)
    valid = (wp >= 0).astype(np.float32)
    wpc = np.clip(wp, 0, None)
    k = wpc // 32
    base = np.minimum(32 * k, W - PW)
    col = PW * (k // 2) + (w - base)  # [D, W]
    pl = np.arange(PAIRS)[:, None, None]
    row = 64 * (k % 2)[None] + 32 * pl + (wpc % 32)[None]  # [2, D, W]
    col2 = np.broadcast_to(col[None], row.shape)

    vol = np.empty((B * G, D, H, W), np.float32)
    for core in range(N_CORES):
        t = bt[core].transpose(1, 0, 2)  # [h, row, x]
        g = t[:, row, col2].astype(np.float32)  # [H, 2, D, W]
        g *= valid[None, None]
        vol[2 * core : 2 * core + 2] = g.transpose(1, 2, 0, 3)
    return vol.reshape(B, G, D, H, W)


def kernel(refimg_fea, targetimg_fea, num_groups, maxdisp):
    assert int(num_groups) == G and int(maxdisp) == D

    nc = _get_module()
    in_maps = _make_in_maps(refimg_fea, targetimg_fea)
    res = run_bass_kernel_spmd(nc, in_maps, core_ids=list(range(N_CORES)))

    return _host_extract(
        np.stack([r["out_bt"] for r in res.results], axis=0)
    )


# revision 16
# speedup vs baseline: 1.0126x; 1.0126x over previous
"""Group-wise correlation cost volume (build_gwc_volume) on 8 trn2 cores.

volume[b,g,d,h,w] = sum_c ref[b,g,c,h,w] * tgt[b,g,c,h,w-d]  (0 where w<d)

Sharding: 16 (b,g) pairs across 8 cores, 2 pairs per core. Each pair is a
contiguous 64-channel slice of the inputs and a contiguous [D,H,W] slab of
the output.

Per (b,g,h) the volume rows are diagonals of the Gram matrix
G[w',w] = sum_c tgt[c,w'] * ref[c,w].  Only the band d = w - w' in [0,48)
is needed, so the Gram is computed as 8 column-piece matmuls (w' pieces of
32), each with an 80-wide moving window R[:, BASE_k : BASE_k+80) written
at a fixed offset of a PSUM bank.

Both (b,g) pairs are fused into ONE K=128 matmul per piece with
block-diagonal zero-padded weights: lhsT[128, 64] holds pair0's 32
tgt-columns in rows 0:64 / cols 0:32 and pair1's in rows 64:128 /
cols 32:64, zeros elsewhere, so the cross-pair products vanish.  The
moving operand is the natural 128-partition ref tile (both pairs
stacked), shared by the two pairs since the window depends only on the
piece.  This halves MATMUL+LDWEIGHTS instruction counts vs per-pair
K=64 matmuls — the weight-load path (one 128-row column per cycle) is
the tensor engine's serial resource for this shape.  The interleaved
weights are built on DVE as uint32 moves; the zero halves live in
persistent SBUF buffers memset once at kernel start.

Two h rows share one 2-bank PSUM tile so each PSUM->SBUF evacuation
instruction covers 2 h of band (amortizes the ~150ns fixed cost per
copy); the evacuation is split 150/170 columns across DVE/ACT to match
their measured per-column rates.  Output DMA rides the otherwise-idle
GPSIMD (SWDGE) queue, input DMA on sync (HWDGE).

Everything runs in bf16 (inputs rounded on the host, PSUM fp32
accumulation, band tiles stored back as bf16): rel err vs the fp32
reference is ~4e-3, well under the 2e-2 gate.

Diagonal (shear) extraction at 1-partition granularity is not expressible
in any engine's access patterns, so the 80-wide band tiles are DMAed out
and the diagonals are gathered on the host during unsharding.
"""

import sys

if "/opt/trn_rl_repo" not in sys.path:
    sys.path.insert(0, "/opt/trn_rl_repo")

import numpy as np
import ml_dtypes

import concourse.bacc as bacc
import concourse.tile as tile
from concourse import mybir
from concourse.bass_utils import run_bass_kernel_spmd

F32 = mybir.dt.float32
BF16 = mybir.dt.bfloat16
U32 = mybir.dt.uint32

B, C, H, W = 2, 512, 128, 256
G, CG, D = 8, 64, 48
N_CORES = 8
PAIRS = 2  # (b,g) pairs per core
HC = 16  # h rows per chunk
PW = 80  # piece window width (32 + 47 + 1)
NW = 4  # persistent interleaved-weight buffers (chunk ring)
XSPLIT = 96  # evacuation column split between DVE [0:XSPLIT] and ACT

# piece k covers w' in [32k, 32k+32); pieces 0-5 get a full 80-wide
# moving window at BASE=32k; pieces 6,7 share a 64-wide window at 192
# (their bands clip at w=255), trimming the stored band to 304 cols.
BASE = [0, 32, 64, 96, 128, 160, 192, 192]
PWK = [80, 80, 80, 80, 80, 80, 64, 64]
C0 = [0, 0, 80, 80, 160, 160, 240, 240]
OUTW = 304

_cached = {}


def _build_module():
    nc = bacc.Bacc("TRN2", target_bir_lowering=False, debug=False, num_devices=N_CORES)
    # io[p, h, 0] = ref channel p, io[p, h, 1] = tgt channel p  (p = pair*64+c)
    io = nc.dram_tensor("io", [128, H, 2, W], BF16, kind="ExternalInput")
    # band tiles, layout [row, h, x]: row p = 64*(k%2) + 32*pair + (w'%32),
    # col x = 80*(k//2) + (w - BASE[k]) for piece k = w'//32.
    out_bt = nc.dram_tensor("out_bt", [128, H, OUTW], BF16, kind="ExternalOutput")

    with tile.TileContext(nc) as tc:
        with (
            tc.tile_pool(name="ins", bufs=5) as ins,
            tc.tile_pool(name="wts", bufs=1) as wts,
            tc.tile_pool(name="stage", bufs=3) as stage_pool,
            tc.tile_pool(name="psum", bufs=2, space="PSUM") as psum,
        ):
            # persistent zero-padded weight buffers: the zero halves are
            # written once here and never touched again (the per-chunk
            # builds only overwrite the data sub-blocks).
            warm = wts.tile([1, 8], BF16, tag="warm", name="warm", bufs=1)
            nc.scalar.copy(warm[0:1, 0:4], warm[0:1, 4:8])

            wt_bufs = []
            for i in range(NW):
                wtb = wts.tile(
                    [128, HC, 8, 64], BF16, tag=f"wt{i}", name=f"wt{i}", bufs=1
                )
                wt_bufs.append(wtb)

            for ch in range(H // HC):
                h0 = ch * HC
                it = ins.tile([128, HC, 2, W], BF16, tag="io", name=f"it{ch}")
                wt = wt_bufs[ch % NW]
                # chunk 0 is loaded/interleaved in 4h slivers so the first
                # matmuls start ~7us earlier (input buffers recycle sooner,
                # keeping the input DMA stream gap-free mid-kernel).
                nsub = 4 if ch == 0 else 1
                hs = HC // nsub
                for q in range(nsub):
                    hq = q * hs
                    nc.sync.dma_start(
                        it[:, hq : hq + hs], io[:, h0 + hq : h0 + hq + hs, :, :]
                    )
                    # zero-prime each persistent weight buffer sliver just
                    # before its first use (a single up-front prime pass
                    # would queue ~9us of DVE memsets ahead of the first
                    # interleave builds and delay the first matmul)
                    if ch < NW:
                        nc.vector.memset(wt[:, hq : hq + hs].bitcast(U32), 0)
                    # interleave tgt into the block-diagonal weight layout
                    # (bf16 pairs moved as uint32 for 2x DVE throughput)
                    tgt_lo = (
                        it[0:64, hq : hq + hs, 1]
                        .rearrange("p hl (k m) -> p hl k m", k=8)
                        .bitcast(U32)
                    )
                    tgt_hi = (
                        it[64:128, hq : hq + hs, 1]
                        .rearrange("p hl (k m) -> p hl k m", k=8)
                        .bitcast(U32)
                    )
                    nc.vector.tensor_copy(
                        wt[0:64, hq : hq + hs, :, 0:32].bitcast(U32), tgt_lo
                    )
                    nc.vector.tensor_copy(
                        wt[64:128, hq : hq + hs, :, 32:64].bitcast(U32), tgt_hi
                    )

                st = stage_pool.tile(
                    [128, HC, OUTW], BF16, tag="st", name=f"st{ch}"
                )
                for hh in range(HC // 4):
                    # one 4-bank PSUM tile holds four h rows of band
                    bank = psum.tile(
                        [128, 4, 512], F32, tag="bk", name=f"bk{ch}_{hh}"
                    )
                    for hl2 in range(4):
                        hl = 4 * hh + hl2
                        for k in range(8):
                            m0 = 64 * (k % 2)
                            nc.tensor.matmul(
                                bank[m0 : m0 + 64, hl2, C0[k] : C0[k] + PWK[k]],
                                wt[:, hl, k, :],
                                it[:, hl, 0, BASE[k] : BASE[k] + PWK[k]],
                                tile_position=(0, m0),
                            )
                    h2 = 4 * hh
                    nc.vector.tensor_copy(
                        st[:, h2 : h2 + 4, 0:XSPLIT], bank[:, :, 0:XSPLIT]
                    )
                    nc.scalar.copy(
                        st[:, h2 : h2 + 4, XSPLIT:OUTW],
                        bank[:, :, XSPLIT:OUTW],
                    )
                    if hh % 2 == 1:
                        # flush each 8h half as soon as its copies land —
                        # smooths the output stream and shortens the drain
                        hf = 4 * (hh - 1)
                        nc.scalar.dma_start(
                            out_bt[:, h0 + hf : h0 + hf + 8, :],
                            st[:, hf : hf + 8],
                        )

    nc.compile()
    return nc


def _get_module():
    if "nc" not in _cached:
        _cached["nc"] = _build_module()
    return _cached["nc"]


def _to_bf16(a):
    """Round-to-nearest-even fp32 -> bf16 without ml_dtypes' slow cast."""
    u = np.ascontiguousarray(a, np.float32).view(np.uint32)
    r = (u + np.uint32(0x7FFF) + ((u >> np.uint32(16)) & np.uint32(1))) >> np.uint32(16)
    return r.astype(np.uint16).view(ml_dtypes.bfloat16)


def _make_in_maps(refimg_fea, targetimg_fea):
    rp = _to_bf16(refimg_fea).reshape(N_CORES, 128, H, W)
    tp = _to_bf16(targetimg_fea).reshape(N_CORES, 128, H, W)
    io = np.ascontiguousarray(np.stack([rp, tp], axis=3))  # [8, 128, H, 2, W]
    return [{"io": io[k]} for k in range(N_CORES)]


def _host_extract(bt):
    """Gather band diagonals into the full volume.

    bt: [8, 128, H, 304] bf16 per core.  For disparity d at width w
    (valid when w >= d): w' = w - d, piece k = w'//32, row
    p = 64*(k%2) + 32*pair_local + (w'%32), col x = 80*(k//2) +
    (w - BASE[k]).
    """
    d = np.arange(D)[:, None]
    w = np.arange(W)[None, :]
    wp = w - d  # [D, W] source w' (negative -> zero region)
    valid = (wp >= 0).astype(np.float32)
    wpc = np.clip(wp, 0, None)
    k = wpc // 32
    base = np.asarray(BASE)[k]
    col = np.asarray(C0)[k] + (w - base)  # [D, W]
    pl = np.arange(PAIRS)[:, None, None]
    row = 64 * (k % 2)[None] + 32 * pl + (wpc % 32)[None]  # [2, D, W]
    col2 = np.broadcast_to(col[None], row.shape)

    vol = np.empty((B * G, D, H, W), np.float32)
    for core in range(N_CORES):
        t = bt[core].transpose(1, 0, 2)  # [h, row, x]
        g = t[:, row, col2].astype(np.float32)  # [H, 2, D, W]
        g *= valid[None, None]
        vol[2 * core : 2 * core + 2] = g.transpose(1, 2, 0, 3)
    return vol.reshape(B, G, D, H, W)


def kernel(refimg_fea, targetimg_fea, num_groups, maxdisp):
    assert int(num_groups) == G and int(maxdisp) == D

    nc = _get_module()
    in_maps = _make_in_maps(refimg_fea, targetimg_fea)
    res = run_bass_kernel_spmd(nc, in_maps, core_ids=list(range(N_CORES)))

    return _host_extract(
        np.stack([r["out_bt"] for r in res.results], axis=0)
    )
